# revision 1
# baseline (speedup 1.0000x reference)
"""CharLSTM (B=128, T=256, V=256, D=1024, L=4) on 8 trn2 NeuronCores.

Strategy: tensor-parallel over the 4*D gate dimension (each core owns a
512-wide slice: 128 columns of each gate, mtile order [i, f, o, g]).
Per time step, each core computes its z slice with bf16 matmuls
(fp32 PSUM accumulation), applies the LSTM cell elementwise for its
128-wide h/c slice, and the 8 h slices (x4 layers) are AllGathered so
every core has the full h vectors for the next step / next layer.
Layers run with a wavefront skew of SKEW ticks so the AllGather latency
hides under independent matmuls of other layers.

Layer-0's x contribution uses a one-hot matmul against G0 = embed @ Wx[0]
(V=256 -> K=256 contraction instead of K=1024), with the one-hot built
on the host from idx (pure int relabeling of the input).

The output projection (h3 @ Wout) is computed redundantly on every core,
batched over 4-step windows (N=512); the host reads core 0's copy.
"""

import numpy as np
import ml_dtypes

B, T, V, D, L = 128, 256, 256, 1024, 4
NCORES = 8
SKEW = 2
BF16 = ml_dtypes.bfloat16

# mtile order [i, f, o, g]; reference gate column blocks are i,f,g,o.
GATE_STARTS = [0, D, 3 * D, 2 * D]


def _gcols(j, m):
    s = GATE_STARTS[m] + j * 128
    return slice(s, s + 128)


def _host_prep(idx, embed, Wx, Wh, b, Wout, t_run):
    """Build per-core input maps (numpy)."""
    nw = t_run // 4
    idx = np.asarray(idx)
    embed = np.asarray(embed, np.float32)
    Wx = np.asarray(Wx, np.float32)
    Wh = np.asarray(Wh, np.float32)
    b = np.asarray(b, np.float32)
    Wout = np.asarray(Wout, np.float32)

    # embt[p, k, v] = embed[v, k*128+p]
    embt = np.ascontiguousarray(
        embed.T.reshape(8, 128, V).transpose(1, 0, 2)).astype(BF16)
    # wout[p, k, c, vv] = Wout[k*128+p, c*128+vv]
    wout = np.ascontiguousarray(
        Wout.reshape(8, 128, 2, 128).transpose(1, 0, 2, 3)).astype(BF16)
    # one-hot: oh[w, p, c, kk, bb] = (idx[bb, 4w+kk] == c*128+p)
    ids = idx[:, :t_run]  # [B, t_run]
    oh = np.zeros((nw, 128, 2, 4, 128), np.float32)
    tt = np.arange(t_run)
    onehot = (ids[None, :, :] == np.arange(V)[:, None, None])  # [V, B, t]
    # -> [c, p, b, w, kk]
    oh_full = onehot.reshape(2, 128, B, nw, 4)
    oh = np.ascontiguousarray(
        oh_full.transpose(3, 1, 0, 4, 2)).astype(BF16)  # [w, p, c, kk, bb]

    in_maps = []
    for j in range(NCORES):
        wx_j = np.empty((L, 128, 8, 4, 128), np.float32)
        wh_j = np.empty((L, 128, 8, 4, 128), np.float32)
        bias_j = np.empty((128, L, 4), np.float32)
        for l in range(L):
            for m in range(4):
                cols = _gcols(j, m)
                # [1024, 128] -> [8, 128p, 128mm] -> assign [p, k, mm]
                wx_j[l, :, :, m, :] = Wx[l][:, cols].reshape(8, 128, 128).transpose(1, 0, 2)
                wh_j[l, :, :, m, :] = Wh[l][:, cols].reshape(8, 128, 128).transpose(1, 0, 2)
                bias_j[:, l, m] = b[l][cols]
        in_maps.append({
            "wx": wx_j.astype(BF16),
            "wh": wh_j.astype(BF16),
            "bias": bias_j,
            "embt": embt,
            "wout": wout,
            "oh": oh,
        })
    return in_maps


def _build(nc, tile, mybir, t_run):
    """Emit the SPMD program for one core (identical on all cores)."""
    dt = mybir.dt
    nw = t_run // 4

    wx_ext = nc.dram_tensor("wx", [L, 128, 8, 4, 128], dt.bfloat16, kind="ExternalInput")
    wh_ext = nc.dram_tensor("wh", [L, 128, 8, 4, 128], dt.bfloat16, kind="ExternalInput")
    bias_ext = nc.dram_tensor("bias", [128, L, 4], dt.float32, kind="ExternalInput")
    embt_ext = nc.dram_tensor("embt", [128, 8, V], dt.bfloat16, kind="ExternalInput")
    wout_ext = nc.dram_tensor("wout", [128, 8, 2, 128], dt.bfloat16, kind="ExternalInput")
    oh_ext = nc.dram_tensor("oh", [nw, 128, 2, 4, 128], dt.bfloat16, kind="ExternalInput")
    out_ext = nc.dram_tensor("logits", [t_run, 128, 2, 128], dt.float32, kind="ExternalOutput")

    rg = [list(range(NCORES))]

    with tile.TileContext(nc) as tc:
        with (
            tc.tile_pool(name="const", bufs=1) as cpool,
            tc.tile_pool(name="state", bufs=1) as spool,
            tc.tile_pool(name="work", bufs=3) as wpool,
            tc.tile_pool(name="ohp", bufs=2) as ohpool,
            tc.tile_pool(name="psum", bufs=8, space="PSUM") as psum,
            tc.tile_pool(name="ccin", bufs=3, space="DRAM") as ccin_pool,
            tc.tile_pool(name="ccout", bufs=3, space="DRAM") as ccout_pool,
        ):
            # ---- resident tiles ----
            wx_s = cpool.tile([128, L, 8, 4, 128], dt.bfloat16)
            wh_s = cpool.tile([128, L, 8, 4, 128], dt.bfloat16)
            bias_s = cpool.tile([128, L, 4], dt.float32)
            embt_s = cpool.tile([128, 8, V], dt.bfloat16)
            wout_s = cpool.tile([128, 8, 2, 128], dt.bfloat16)
            g0_s = cpool.tile([128, 2, 4, 128], dt.bfloat16)
            hbuf = spool.tile([128, L, 8, 8, 128], dt.bfloat16)  # (l, kchunk, slot, b)
            c_s = spool.tile([128, L, 128], dt.float32)

            for l in range(L):
                nc.sync.dma_start(wx_s[:, l], wx_ext[l])
                nc.sync.dma_start(wh_s[:, l], wh_ext[l])
            nc.sync.dma_start(bias_s[:], bias_ext[:])
            nc.sync.dma_start(embt_s[:], embt_ext[:])
            nc.sync.dma_start(wout_s[:], wout_ext[:])

            # h(-1) = 0
            nc.vector.memset(hbuf[:, :, :, 7, :], 0.0)

            # ---- G0 = embed @ Wx[0] (slice), bf16 ----
            for c in range(2):
                pg = psum.tile([128, 512], dt.float32, tag="ps")
                for k in range(8):
                    nc.tensor.matmul(
                        pg[:],
                        embt_s[:, k, c * 128:(c + 1) * 128],
                        wx_s[:, 0, k],
                        start=(k == 0), stop=(k == 7),
                    )
                nc.vector.tensor_copy(g0_s[:, c], pg[:].rearrange("p (m n) -> p m n", m=4))

            # ---- main loop over ticks ----
            total_ticks = t_run + SKEW * (L - 1)
            oh_tiles = {}
            for tau in range(total_ticks):
                active = [(l, tau - SKEW * l) for l in range(L)]
                active = [(l, t) for (l, t) in active if 0 <= t < t_run]
                act_map = dict(active)

                ccinA = ccin_pool.tile([2, 128, 128], dt.bfloat16, name="ccinA")
                ccinB = ccin_pool.tile([2, 128, 128], dt.bfloat16, name="ccinB")
                ccoutA = ccout_pool.tile([NCORES, 2, 128, 128], dt.bfloat16,
                                         addr_space="Shared", name="ccoutA")
                ccoutB = ccout_pool.tile([NCORES, 2, 128, 128], dt.bfloat16,
                                         addr_space="Shared", name="ccoutB")

                # one-hot window prefetch for layer 0 (2 ticks ahead of use)
                t0 = tau  # layer 0 runs unskewed
                for w in range(nw):
                    if max(0, w * 4 - 2) == tau:
                        oht = ohpool.tile([128, 2, 4, 128], dt.bfloat16, name="oh_t")
                        nc.sync.dma_start(oht[:], oh_ext[w])
                        oh_tiles[w] = oht
                oh_t = oh_tiles.get(t0 // 4)

                # ---- phase 1: x-part matmuls of ALL active layers first ----
                # (they depend only on gathers >= 2 ticks old, so the PE can
                # run them while this tick's AllGathers are still in flight)
                zps = {}
                for l, t in active:
                    zp = psum.tile([128, 512], dt.float32, name="zp", tag="ps")
                    zps[l] = zp
                    for m in range(4):
                        if l == 0:
                            xmms = [(g0_s[:, c, m], oh_t[:, c, t % 4, :])
                                    for c in range(2)]
                        else:
                            xmms = [(wx_s[:, l, k, m], hbuf[:, l - 1, k, t % 8, :])
                                    for k in range(8)]
                        n = len(xmms)
                        for i, (lhsT, rhs) in enumerate(xmms):
                            nc.tensor.matmul(
                                zp[:, m * 128:(m + 1) * 128], lhsT, rhs,
                                start=(i == 0), stop=(t == 0 and i == n - 1),
                            )

                # ---- phase 2: per layer: h-part matmuls + cell + stage ----
                lastA = max([l for l, _ in active if l < 2], default=None)
                lastB = max([l for l, _ in active if l >= 2], default=None)
                for l, t in active:
                    zp = zps[l]
                    if t > 0:
                        for m in range(4):
                            for k in range(8):
                                nc.tensor.matmul(
                                    zp[:, m * 128:(m + 1) * 128],
                                    wh_s[:, l, k, m], hbuf[:, l, k, (t - 1) % 8, :],
                                    start=False, stop=(k == 7),
                                )

                    # ---- LSTM cell elementwise (z^T layout: partition = gate dim) ----
                    sig = wpool.tile([128, 384], dt.float32, name="sig")
                    gth = wpool.tile([128, 128], dt.float32, name="gth")
                    for m in range(3):  # i, f, o with per-gate bias
                        nc.scalar.activation(
                            sig[:, m * 128:(m + 1) * 128], zp[:, m * 128:(m + 1) * 128],
                            mybir.ActivationFunctionType.Sigmoid,
                            bias=bias_s[:, l, m:m + 1],
                        )
                    nc.scalar.activation(
                        gth[:], zp[:, 384:512],
                        mybir.ActivationFunctionType.Tanh,
                        bias=bias_s[:, l, 3:4],
                    )
                    ig = wpool.tile([128, 128], dt.float32, name="ig")
                    nc.vector.tensor_mul(ig[:], sig[:, 0:128], gth[:])
                    cv = c_s[:, l]
                    if t > 0:
                        nc.vector.tensor_mul(cv, cv, sig[:, 128:256])
                        nc.vector.tensor_add(cv, cv, ig[:])
                    else:
                        nc.vector.tensor_copy(cv, ig[:])
                    tch = wpool.tile([128, 128], dt.float32, name="tch")
                    nc.scalar.activation(tch[:], cv, mybir.ActivationFunctionType.Tanh)
                    h_sl = wpool.tile([128, 128], dt.bfloat16, name="h_sl")
                    nc.vector.tensor_mul(h_sl[:], sig[:, 256:384], tch[:])
                    nc.sync.dma_start((ccinA if l < 2 else ccinB)[l % 2], h_sl[:])

                    # post the pair AllGather as soon as its two layers are done
                    if l == lastA or l == lastB:
                        ccin_p = ccinA if l == lastA else ccinB
                        ccout_p = ccoutA if l == lastA else ccoutB
                        nc.gpsimd.collective_compute(
                            "AllGather", mybir.AluOpType.bypass,
                            replica_groups=rg, ins=[ccin_p[:]], outs=[ccout_p[:]],
                        )
                        for lp in ((0, 1) if l == lastA else (2, 3)):
                            if lp not in act_map:
                                continue
                            tp = act_map[lp]
                            # ccout[r, lp%2, p, b] -> hbuf[p, lp, r, slot, b]
                            nc.sync.dma_start(
                                hbuf[:, lp, :, tp % 8, :],
                                ccout_p[:, lp % 2, :, :].transpose((1, 0, 2)),
                            )

                # ---- projection of layer-3 output, per tick (1 PSUM bank) ----
                t3 = tau - SKEW * 3
                if 0 <= t3 < t_run:
                    pp = psum.tile([128, 256], dt.float32, name="pp", tag="ps")
                    for c in range(2):
                        for k in range(8):
                            nc.tensor.matmul(
                                pp[:, c * 128:(c + 1) * 128],
                                wout_s[:, k, c], hbuf[:, 3, k, t3 % 8, :],
                                start=(k == 0), stop=(k == 7),
                            )
                    lg = wpool.tile([128, 256], dt.float32, name="lg")
                    nc.vector.tensor_copy(lg[:], pp[:])
                    nc.sync.dma_start(out_ext[t3], lg[:].rearrange("p (c b) -> p c b", c=2))

    nc.compile()
    return nc


_CACHED = {}


def _get_nc(t_run):
    if t_run in _CACHED:
        return _CACHED[t_run]
    import concourse.bass as bass  # noqa: PLC0415
    import concourse.tile as tile  # noqa: PLC0415
    from concourse import bacc, mybir  # noqa: PLC0415

    nc = bacc.Bacc("TRN2", target_bir_lowering=False, debug=False,
                   num_devices=NCORES)
    _build(nc, tile, mybir, t_run)
    _CACHED[t_run] = nc
    return nc


def kernel(idx, embed, Wx, Wh, b, Wout, _t_run=T):
    from concourse.bass_utils import run_bass_kernel_spmd  # noqa: PLC0415

    t_run = _t_run
    in_maps = _host_prep(idx, embed, Wx, Wh, b, Wout, t_run)
    nc = _get_nc(t_run)
    res = run_bass_kernel_spmd(nc, in_maps, core_ids=list(range(NCORES)))
    out = res.results[0]["logits"]  # [t, vv, c, b]
    logits = np.ascontiguousarray(
        out.transpose(3, 0, 2, 1).reshape(B, t_run, V)).astype(np.float32)
    return logits



# revision 2
# speedup vs baseline: 1.4255x; 1.4255x over previous
"""CharLSTM (B=128, T=256, V=256, D=1024, L=4) on 8 trn2 NeuronCores.

Strategy: tensor-parallel over the 4*D gate dimension (each core owns a
512-wide slice: 128 columns of each gate, mtile order [i, f, o, g]).
Per time step, each core computes its z slice with bf16 matmuls
(fp32 PSUM accumulation), applies the LSTM cell elementwise for its
128-wide h/c slice, and the 8 h slices are AllGathered (in two
layer-pairs per tick) so every core has the full h vectors for the
next step / next layer.  Layers run with a wavefront skew of SKEW
ticks so the AllGather latency hides under independent matmuls.

Gathered h state lives in per-(layer, slot) SBUF tiles (8 rotating
slots indexed by t%8) so the Tile dependency tracker sees the
x-part matmuls of tick N+1 as independent of tick N's AllGather --
a single big h buffer falsely serializes every tick behind the
newest gather.

Layer-0's x contribution uses a one-hot matmul against G0 = embed @ Wx[0]
(V=256 -> K=256 contraction instead of K=1024), with the one-hot built
on the host from idx (pure int relabeling of the input).

The output projection (h3 @ Wout) is sharded over the vocab dim:
each core holds a 32-column slice of Wout and projects every step
(2-tick delayed so it only uses settled gathers); the host
concatenates the 8 vocab slices.
"""

import numpy as np
import ml_dtypes

B, T, V, D, L = 128, 256, 256, 1024, 4
NCORES = 8
SKEW = 2
VSL = V // NCORES  # per-core vocab slice (32)
PROJ_DELAY = SKEW * 3 + 2  # proj of step t runs at tick t + PROJ_DELAY
BF16 = ml_dtypes.bfloat16

# mtile order [i, f, o, g]; reference gate column blocks are i,f,g,o.
GATE_STARTS = [0, D, 3 * D, 2 * D]


def _gcols(j, m):
    s = GATE_STARTS[m] + j * 128
    return slice(s, s + 128)


def _host_prep(idx, embed, Wx, Wh, b, Wout, t_run):
    """Build per-core input maps (numpy)."""
    nw = t_run // 4
    idx = np.asarray(idx)
    embed = np.asarray(embed, np.float32)
    Wx = np.asarray(Wx, np.float32)
    Wh = np.asarray(Wh, np.float32)
    b = np.asarray(b, np.float32)
    Wout = np.asarray(Wout, np.float32)

    # embt[p, k, v] = embed[v, k*128+p]
    embt = np.ascontiguousarray(
        embed.T.reshape(8, 128, V).transpose(1, 0, 2)).astype(BF16)
    # one-hot: oh[w, p, c, kk, bb] = (idx[bb, 4w+kk] == c*128+p)
    ids = idx[:, :t_run]  # [B, t_run]
    onehot = (ids[None, :, :] == np.arange(V)[:, None, None])  # [V, B, t]
    oh_full = onehot.reshape(2, 128, B, nw, 4)
    oh = np.ascontiguousarray(
        oh_full.transpose(3, 1, 0, 4, 2)).astype(BF16)  # [w, p, c, kk, bb]

    in_maps = []
    for j in range(NCORES):
        wx_j = np.empty((L, 128, 8, 4, 128), np.float32)
        wh_j = np.empty((L, 128, 8, 4, 128), np.float32)
        bias_j = np.empty((128, L, 4), np.float32)
        for l in range(L):
            for m in range(4):
                cols = _gcols(j, m)
                # [1024, 128] -> [8, 128p, 128mm] -> assign [p, k, mm]
                wx_j[l, :, :, m, :] = Wx[l][:, cols].reshape(8, 128, 128).transpose(1, 0, 2)
                wh_j[l, :, :, m, :] = Wh[l][:, cols].reshape(8, 128, 128).transpose(1, 0, 2)
                bias_j[:, l, m] = b[l][cols]
        # wout[p, k, vv] = Wout[k*128+p, j*32+vv]
        wout_j = np.ascontiguousarray(
            Wout[:, j * VSL:(j + 1) * VSL].reshape(8, 128, VSL).transpose(1, 0, 2)
        ).astype(BF16)
        in_maps.append({
            "wx": wx_j.astype(BF16),
            "wh": wh_j.astype(BF16),
            "bias": bias_j,
            "embt": embt,
            "wout": wout_j,
            "oh": oh,
        })
    return in_maps


def _build(nc, tile, mybir, t_run):
    """Emit the SPMD program for one core (identical on all cores)."""
    dt = mybir.dt
    nw = t_run // 4

    wx_ext = nc.dram_tensor("wx", [L, 128, 8, 4, 128], dt.bfloat16, kind="ExternalInput")
    wh_ext = nc.dram_tensor("wh", [L, 128, 8, 4, 128], dt.bfloat16, kind="ExternalInput")
    bias_ext = nc.dram_tensor("bias", [128, L, 4], dt.float32, kind="ExternalInput")
    embt_ext = nc.dram_tensor("embt", [128, 8, V], dt.bfloat16, kind="ExternalInput")
    wout_ext = nc.dram_tensor("wout", [128, 8, VSL], dt.bfloat16, kind="ExternalInput")
    oh_ext = nc.dram_tensor("oh", [nw, 128, 2, 4, 128], dt.bfloat16, kind="ExternalInput")
    out_ext = nc.dram_tensor("logits", [t_run, VSL, 128], dt.float32, kind="ExternalOutput")

    rg = [list(range(NCORES))]

    with tile.TileContext(nc) as tc:
        with (
            tc.tile_pool(name="const", bufs=1) as cpool,
            tc.tile_pool(name="state", bufs=1) as spool,
            tc.tile_pool(name="work", bufs=6) as wpool,
            tc.tile_pool(name="ohp", bufs=2) as ohpool,
            tc.tile_pool(name="zpsum", bufs=6, space="PSUM") as zpsum,
            tc.tile_pool(name="ppsum", bufs=2, space="PSUM") as ppsum,
            tc.tile_pool(name="ccin", bufs=3, space="DRAM") as ccin_pool,
            tc.tile_pool(name="ccout", bufs=3, space="DRAM") as ccout_pool,
        ):
            # ---- resident tiles ----
            wx_s = cpool.tile([128, L, 8, 4, 128], dt.bfloat16)
            wh_s = cpool.tile([128, L, 8, 4, 128], dt.bfloat16)
            bias_s = cpool.tile([128, L, 4], dt.float32)
            embt_s = cpool.tile([128, 8, V], dt.bfloat16)
            wout_s = cpool.tile([128, 8, VSL], dt.bfloat16)
            g0_s = cpool.tile([128, 2, 4, 128], dt.bfloat16)
            # gathered h state: separate tile per (layer, slot) so tick N+1's
            # x-part reads (old slots) don't serialize behind tick N's fill.
            hb = [[cpool.tile([128, 8, 128], dt.bfloat16, name=f"hb{l}_{s}")
                   for s in range(8)] for l in range(L)]
            c_s = [spool.tile([128, 128], dt.float32, name=f"c{l}") for l in range(L)]

            for l in range(L):
                nc.sync.dma_start(wx_s[:, l], wx_ext[l])
                nc.sync.dma_start(wh_s[:, l], wh_ext[l])
            nc.sync.dma_start(bias_s[:], bias_ext[:])
            nc.sync.dma_start(embt_s[:], embt_ext[:])
            nc.sync.dma_start(wout_s[:], wout_ext[:])

            # h(-1) = 0 lives in slot 7 (t=-1 mod 8)
            for l in range(L):
                nc.vector.memset(hb[l][7][:], 0.0)

            # ---- G0 = embed @ Wx[0] (slice), bf16 ----
            for c in range(2):
                pg = zpsum.tile([128, 512], dt.float32, tag="zp")
                for k in range(8):
                    nc.tensor.matmul(
                        pg[:],
                        embt_s[:, k, c * 128:(c + 1) * 128],
                        wx_s[:, 0, k],
                        start=(k == 0), stop=(k == 7),
                    )
                nc.vector.tensor_copy(g0_s[:, c], pg[:].rearrange("p (m n) -> p m n", m=4))

            # ---- main loop over ticks ----
            total_ticks = t_run + PROJ_DELAY
            oh_tiles = {}
            for tau in range(total_ticks):
                active = [(l, tau - SKEW * l) for l in range(L)]
                active = [(l, t) for (l, t) in active if 0 <= t < t_run]
                act_map = dict(active)

                # one-hot window prefetch for layer 0 (2 ticks ahead of use)
                for w in range(nw):
                    if max(0, w * 4 - 2) == tau:
                        oht = ohpool.tile([128, 2, 4, 128], dt.bfloat16, name="oh_t")
                        nc.sync.dma_start(oht[:], oh_ext[w])
                        oh_tiles[w] = oht
                oh_t = oh_tiles.get(tau // 4)

                # ---- projection of h3 (settled gathers only) ----
                tp = tau - PROJ_DELAY
                if 0 <= tp < t_run:
                    pp = ppsum.tile([VSL, 128], dt.float32, name="pp", tag="pp")
                    hsrc = hb[3][tp % 8]
                    for k in range(8):
                        nc.tensor.matmul(
                            pp[:], wout_s[:, k], hsrc[:, k, :],
                            start=(k == 0), stop=(k == 7),
                        )
                    lg = wpool.tile([VSL, 128], dt.float32, name="lg")
                    nc.vector.tensor_copy(lg[:], pp[:])
                    nc.sync.dma_start(out_ext[tp], lg[:])

                # ---- phase 1: x-part matmuls of ALL active layers ----
                # (depend only on gathers >= 2 ticks old)
                zps = {}
                for l, t in active:
                    zp = zpsum.tile([128, 512], dt.float32, name="zp", tag="zp")
                    zps[l] = zp
                    for m in range(4):
                        if l == 0:
                            xmms = [(g0_s[:, c, m], oh_t[:, c, t % 4, :])
                                    for c in range(2)]
                        else:
                            hsrc = hb[l - 1][t % 8]
                            xmms = [(wx_s[:, l, k, m], hsrc[:, k, :])
                                    for k in range(8)]
                        n = len(xmms)
                        for i, (lhsT, rhs) in enumerate(xmms):
                            nc.tensor.matmul(
                                zp[:, m * 128:(m + 1) * 128], lhsT, rhs,
                                start=(i == 0), stop=(t == 0 and i == n - 1),
                            )

                # ---- phase 2: per layer: h-part matmuls + cell + stage ----
                lastA = max([l for l, _ in active if l < 2], default=None)
                lastB = max([l for l, _ in active if l >= 2], default=None)
                ccinA = ccin_pool.tile([2, 128, 128], dt.bfloat16, name="ccinA")
                ccinB = ccin_pool.tile([2, 128, 128], dt.bfloat16, name="ccinB")
                ccoutA = ccout_pool.tile([NCORES, 2, 128, 128], dt.bfloat16,
                                         addr_space="Shared", name="ccoutA")
                ccoutB = ccout_pool.tile([NCORES, 2, 128, 128], dt.bfloat16,
                                         addr_space="Shared", name="ccoutB")

                for l, t in active:
                    zp = zps[l]
                    if t > 0:
                        hsrc = hb[l][(t - 1) % 8]
                        for m in range(4):
                            for k in range(8):
                                nc.tensor.matmul(
                                    zp[:, m * 128:(m + 1) * 128],
                                    wh_s[:, l, k, m], hsrc[:, k, :],
                                    start=False, stop=(k == 7),
                                )

                    # ---- LSTM cell elementwise (z^T layout: partition = gate dim) ----
                    sig = wpool.tile([128, 384], dt.float32, name="sig")
                    gth = wpool.tile([128, 128], dt.float32, name="gth")
                    for m in range(3):  # i, f, o with per-gate bias
                        nc.scalar.activation(
                            sig[:, m * 128:(m + 1) * 128], zp[:, m * 128:(m + 1) * 128],
                            mybir.ActivationFunctionType.Sigmoid,
                            bias=bias_s[:, l, m:m + 1],
                        )
                    nc.scalar.activation(
                        gth[:], zp[:, 384:512],
                        mybir.ActivationFunctionType.Tanh,
                        bias=bias_s[:, l, 3:4],
                    )
                    ig = wpool.tile([128, 128], dt.float32, name="ig")
                    nc.vector.tensor_mul(ig[:], sig[:, 0:128], gth[:])
                    cv = c_s[l]
                    if t > 0:
                        nc.vector.tensor_mul(cv[:], cv[:], sig[:, 128:256])
                        nc.vector.tensor_add(cv[:], cv[:], ig[:])
                    else:
                        nc.vector.tensor_copy(cv[:], ig[:])
                    tch = wpool.tile([128, 128], dt.float32, name="tch")
                    nc.scalar.activation(tch[:], cv[:], mybir.ActivationFunctionType.Tanh)
                    h_sl = wpool.tile([128, 128], dt.bfloat16, name="h_sl")
                    nc.vector.tensor_mul(h_sl[:], sig[:, 256:384], tch[:])
                    nc.sync.dma_start((ccinA if l < 2 else ccinB)[l % 2], h_sl[:])

                    # post the pair AllGather as soon as its two layers are done
                    if l == lastA or l == lastB:
                        ccin_p = ccinA if l == lastA else ccinB
                        ccout_p = ccoutA if l == lastA else ccoutB
                        nc.gpsimd.collective_compute(
                            "AllGather", mybir.AluOpType.bypass,
                            replica_groups=rg, ins=[ccin_p[:]], outs=[ccout_p[:]],
                        )
                        for lp in ((0, 1) if l == lastA else (2, 3)):
                            if lp not in act_map:
                                continue
                            tl = act_map[lp]
                            # ccout[r, lp%2, p, b] -> hb[lp][tl%8][p, r, b]
                            nc.gpsimd.dma_start(
                                hb[lp][tl % 8][:],
                                ccout_p[:, lp % 2, :, :].transpose((1, 0, 2)),
                            )

    nc.compile()
    return nc


_CACHED = {}


def _get_nc(t_run):
    if t_run in _CACHED:
        return _CACHED[t_run]
    import concourse.bass as bass  # noqa: PLC0415
    import concourse.tile as tile  # noqa: PLC0415
    from concourse import bacc, mybir  # noqa: PLC0415

    nc = bacc.Bacc("TRN2", target_bir_lowering=False, debug=False,
                   num_devices=NCORES)
    _build(nc, tile, mybir, t_run)
    _CACHED[t_run] = nc
    return nc


def _assemble(results, t_run):
    """results[j]["logits"]: [t, VSL, b] -> full [B, t_run, V]."""
    logits = np.empty((B, t_run, V), np.float32)
    for j in range(NCORES):
        out = np.asarray(results[j]["logits"], np.float32)  # [t, vv, b]
        logits[:, :, j * VSL:(j + 1) * VSL] = out.transpose(2, 0, 1)
    return logits


def kernel(idx, embed, Wx, Wh, b, Wout, _t_run=T):
    from concourse.bass_utils import run_bass_kernel_spmd  # noqa: PLC0415

    t_run = _t_run
    in_maps = _host_prep(idx, embed, Wx, Wh, b, Wout, t_run)
    nc = _get_nc(t_run)
    res = run_bass_kernel_spmd(nc, in_maps, core_ids=list(range(NCORES)))
    return _assemble(res.results, t_run)


# revision 3
# speedup vs baseline: 1.6605x; 1.1649x over previous
"""CharLSTM (B=128, T=256, V=256, D=1024, L=4) on 8 trn2 NeuronCores.

Strategy: tensor-parallel over the 4*D gate dimension (each core owns a
512-wide slice: 128 columns of each gate, mtile order [i, f, o, g]).
Per time step, each core computes its z slice with bf16 matmuls
(fp32 PSUM accumulation), applies the LSTM cell elementwise for its
128-wide h/c slice, and the 8 h slices are AllGathered (in two
layer-pairs per tick) so every core has the full h vectors for the
next step / next layer.  Layers run with a wavefront skew of SKEW
ticks so the AllGather latency hides under independent matmuls.

Gathered h state lives in per-(layer, slot) SBUF tiles (8 rotating
slots indexed by t%8) so the Tile dependency tracker sees the
x-part matmuls of tick N+1 as independent of tick N's AllGather --
a single big h buffer falsely serializes every tick behind the
newest gather.

Layer-0's x contribution uses a one-hot matmul against G0 = embed @ Wx[0]
(V=256 -> K=256 contraction instead of K=1024), with the one-hot built
on the host from idx (pure int relabeling of the input).

The output projection (h3 @ Wout) is sharded over the vocab dim:
each core holds a 32-column slice of Wout and projects every step
(2-tick delayed so it only uses settled gathers); the host
concatenates the 8 vocab slices.
"""

import numpy as np
import ml_dtypes

B, T, V, D, L = 128, 256, 256, 1024, 4
NCORES = 8
SKEW = 2
VSL = V // NCORES  # per-core vocab slice (32)
PROJ_DELAY = SKEW * 3 + 2  # proj of step t runs at tick t + PROJ_DELAY
BF16 = ml_dtypes.bfloat16

# mtile order [i, f, o, g]; reference gate column blocks are i,f,g,o.
GATE_STARTS = [0, D, 3 * D, 2 * D]


def _gcols(j, m):
    s = GATE_STARTS[m] + j * 128
    return slice(s, s + 128)


def _host_prep(idx, embed, Wx, Wh, b, Wout, t_run):
    """Build per-core input maps (numpy)."""
    nw = t_run // 4
    idx = np.asarray(idx)
    embed = np.asarray(embed, np.float32)
    Wx = np.asarray(Wx, np.float32)
    Wh = np.asarray(Wh, np.float32)
    b = np.asarray(b, np.float32)
    Wout = np.asarray(Wout, np.float32)

    # embt[p, k, v] = embed[v, k*128+p]
    embt = np.ascontiguousarray(
        embed.T.reshape(8, 128, V).transpose(1, 0, 2)).astype(BF16)
    # one-hot: oh[w, p, c, kk, bb] = (idx[bb, 4w+kk] == c*128+p)
    ids = idx[:, :t_run]  # [B, t_run]
    onehot = (ids[None, :, :] == np.arange(V)[:, None, None])  # [V, B, t]
    oh_full = onehot.reshape(2, 128, B, nw, 4)
    oh = np.ascontiguousarray(
        oh_full.transpose(3, 1, 0, 4, 2)).astype(BF16)  # [w, p, c, kk, bb]

    in_maps = []
    for j in range(NCORES):
        wx_j = np.empty((L, 128, 8, 4, 128), np.float32)
        wh_j = np.empty((L, 128, 8, 4, 128), np.float32)
        bias_j = np.empty((128, L, 4), np.float32)
        for l in range(L):
            for m in range(4):
                cols = _gcols(j, m)
                # [1024, 128] -> [8, 128p, 128mm] -> assign [p, k, mm]
                wx_j[l, :, :, m, :] = Wx[l][:, cols].reshape(8, 128, 128).transpose(1, 0, 2)
                wh_j[l, :, :, m, :] = Wh[l][:, cols].reshape(8, 128, 128).transpose(1, 0, 2)
                bias_j[:, l, m] = b[l][cols]
        # wout[p, k, vv] = Wout[k*128+p, j*32+vv]
        wout_j = np.ascontiguousarray(
            Wout[:, j * VSL:(j + 1) * VSL].reshape(8, 128, VSL).transpose(1, 0, 2)
        ).astype(BF16)
        in_maps.append({
            "wx": wx_j.astype(BF16),
            "wh": wh_j.astype(BF16),
            "bias": bias_j,
            "embt": embt,
            "wout": wout_j,
            "oh": oh,
        })
    return in_maps


def _build(nc, tile, mybir, t_run):
    """Emit the SPMD program for one core (identical on all cores)."""
    dt = mybir.dt
    nw = t_run // 4

    wx_ext = nc.dram_tensor("wx", [L, 128, 8, 4, 128], dt.bfloat16, kind="ExternalInput")
    wh_ext = nc.dram_tensor("wh", [L, 128, 8, 4, 128], dt.bfloat16, kind="ExternalInput")
    bias_ext = nc.dram_tensor("bias", [128, L, 4], dt.float32, kind="ExternalInput")
    embt_ext = nc.dram_tensor("embt", [128, 8, V], dt.bfloat16, kind="ExternalInput")
    wout_ext = nc.dram_tensor("wout", [128, 8, VSL], dt.bfloat16, kind="ExternalInput")
    oh_ext = nc.dram_tensor("oh", [nw, 128, 2, 4, 128], dt.bfloat16, kind="ExternalInput")
    out_ext = nc.dram_tensor("logits", [t_run, VSL, 128], dt.float32, kind="ExternalOutput")

    rg = [list(range(NCORES))]

    with tile.TileContext(nc) as tc:
        with (
            tc.tile_pool(name="const", bufs=1) as cpool,
            tc.tile_pool(name="state", bufs=1) as spool,
            tc.tile_pool(name="work", bufs=6) as wpool,
            tc.tile_pool(name="ohp", bufs=2) as ohpool,
            tc.tile_pool(name="zpsum", bufs=6, space="PSUM") as zpsum,
            tc.tile_pool(name="ppsum", bufs=2, space="PSUM") as ppsum,
            tc.tile_pool(name="ccin", bufs=3, space="DRAM") as ccin_pool,
            tc.tile_pool(name="ccout", bufs=3, space="DRAM") as ccout_pool,
        ):
            # ---- resident tiles ----
            wx_s = cpool.tile([128, L, 8, 4, 128], dt.bfloat16)
            wh_s = cpool.tile([128, L, 8, 4, 128], dt.bfloat16)
            bias_s = cpool.tile([128, L, 4], dt.float32)
            embt_s = cpool.tile([128, 8, V], dt.bfloat16)
            wout_s = cpool.tile([128, 8, VSL], dt.bfloat16)
            g0_s = cpool.tile([128, 2, 4, 128], dt.bfloat16)
            # gathered h state: separate tile per (layer, slot) so tick N+1's
            # x-part reads (old slots) don't serialize behind tick N's fill.
            hb = [[cpool.tile([128, 8, 128], dt.bfloat16, name=f"hb{l}_{s}")
                   for s in range(8)] for l in range(L)]
            c_s = [spool.tile([128, 128], dt.float32, name=f"c{l}") for l in range(L)]

            for l in range(L):
                nc.sync.dma_start(wx_s[:, l], wx_ext[l])
                nc.sync.dma_start(wh_s[:, l], wh_ext[l])
            nc.sync.dma_start(bias_s[:], bias_ext[:])
            nc.sync.dma_start(embt_s[:], embt_ext[:])
            nc.sync.dma_start(wout_s[:], wout_ext[:])

            # h(-1) = 0 lives in slot 7 (t=-1 mod 8)
            for l in range(L):
                nc.vector.memset(hb[l][7][:], 0.0)

            # ---- G0 = embed @ Wx[0] (slice), bf16 ----
            for c in range(2):
                pg = zpsum.tile([128, 512], dt.float32, tag="zp")
                for k in range(8):
                    nc.tensor.matmul(
                        pg[:],
                        embt_s[:, k, c * 128:(c + 1) * 128],
                        wx_s[:, 0, k],
                        start=(k == 0), stop=(k == 7),
                    )
                nc.vector.tensor_copy(g0_s[:, c], pg[:].rearrange("p (m n) -> p m n", m=4))

            # ---- main loop over ticks ----
            total_ticks = t_run + PROJ_DELAY
            oh_tiles = {}
            for tau in range(total_ticks):
                active = [(l, tau - SKEW * l) for l in range(L)]
                active = [(l, t) for (l, t) in active if 0 <= t < t_run]
                act_map = dict(active)

                # one-hot window prefetch for layer 0 (2 ticks ahead of use)
                for w in range(nw):
                    if max(0, w * 4 - 2) == tau:
                        oht = ohpool.tile([128, 2, 4, 128], dt.bfloat16, name="oh_t")
                        nc.sync.dma_start(oht[:], oh_ext[w])
                        oh_tiles[w] = oht
                oh_t = oh_tiles.get(tau // 4)

                # ---- projection of h3 (settled gathers only) ----
                tp = tau - PROJ_DELAY
                if 0 <= tp < t_run:
                    pp = ppsum.tile([VSL, 128], dt.float32, name="pp", tag="pp")
                    hsrc = hb[3][tp % 8]
                    for k in range(8):
                        nc.tensor.matmul(
                            pp[:], wout_s[:, k], hsrc[:, k, :],
                            start=(k == 0), stop=(k == 7),
                        )
                    lg = wpool.tile([VSL, 128], dt.float32, name="lg")
                    nc.vector.tensor_copy(lg[:], pp[:])
                    nc.sync.dma_start(out_ext[tp], lg[:])

                # ---- phase 1: x-part matmuls of ALL active layers ----
                # (depend only on gathers >= 2 ticks old)
                zps = {}
                for l, t in active:
                    zp = zpsum.tile([128, 512], dt.float32, name="zp", tag="zp")
                    zps[l] = zp
                    for m in range(4):
                        if l == 0:
                            xmms = [(g0_s[:, c, m], oh_t[:, c, t % 4, :])
                                    for c in range(2)]
                        else:
                            hsrc = hb[l - 1][t % 8]
                            xmms = [(wx_s[:, l, k, m], hsrc[:, k, :])
                                    for k in range(8)]
                        n = len(xmms)
                        for i, (lhsT, rhs) in enumerate(xmms):
                            nc.tensor.matmul(
                                zp[:, m * 128:(m + 1) * 128], lhsT, rhs,
                                start=(i == 0), stop=(t == 0 and i == n - 1),
                            )

                # ---- phase 2: per layer: h-part matmuls + cell + stage ----
                # pair B (layers 2,3) first so its AllGather posts early; the
                # other pair's h-matmuls + next tick's x-parts cover each
                # gather's flight time symmetrically.
                ccinA = ccin_pool.tile([2, 128, 128], dt.bfloat16, name="ccinA")
                ccinB = ccin_pool.tile([2, 128, 128], dt.bfloat16, name="ccinB")
                ccoutA = ccout_pool.tile([NCORES, 2, 128, 128], dt.bfloat16,
                                         addr_space="Shared", name="ccoutA")
                ccoutB = ccout_pool.tile([NCORES, 2, 128, 128], dt.bfloat16,
                                         addr_space="Shared", name="ccoutB")
                pairs = [((2, 3), ccinB, ccoutB), ((0, 1), ccinA, ccoutA)]
                posted = []

                for pair, ccin_p, ccout_p in pairs:
                    plast = max([l for l in pair if l in act_map], default=None)
                    for l in pair:
                        if l not in act_map:
                            continue
                        t = act_map[l]
                        zp = zps[l]
                        if t > 0:
                            hsrc = hb[l][(t - 1) % 8]
                            for m in range(4):
                                for k in range(8):
                                    nc.tensor.matmul(
                                        zp[:, m * 128:(m + 1) * 128],
                                        wh_s[:, l, k, m], hsrc[:, k, :],
                                        start=False, stop=(k == 7),
                                    )

                        # ---- LSTM cell (z^T layout: partition = gate dim) ----
                        # mtile gate order [i, f, o, g]; ACT order f,i,g,o
                        # shortens the c-update critical chain.
                        sig = wpool.tile([128, 384], dt.float32, name="sig")
                        gth = wpool.tile([128, 128], dt.float32, name="gth")
                        cv = c_s[l]
                        if t > 0:
                            nc.scalar.activation(
                                sig[:, 128:256], zp[:, 128:256],
                                mybir.ActivationFunctionType.Sigmoid,
                                bias=bias_s[:, l, 1:2],
                            )
                        nc.scalar.activation(
                            sig[:, 0:128], zp[:, 0:128],
                            mybir.ActivationFunctionType.Sigmoid,
                            bias=bias_s[:, l, 0:1],
                        )
                        nc.scalar.activation(
                            gth[:], zp[:, 384:512],
                            mybir.ActivationFunctionType.Tanh,
                            bias=bias_s[:, l, 3:4],
                        )
                        nc.scalar.activation(
                            sig[:, 256:384], zp[:, 256:384],
                            mybir.ActivationFunctionType.Sigmoid,
                            bias=bias_s[:, l, 2:3],
                        )
                        ig = wpool.tile([128, 128], dt.float32, name="ig")
                        nc.vector.tensor_mul(ig[:], sig[:, 0:128], gth[:])
                        if t > 0:
                            nc.vector.tensor_mul(cv[:], cv[:], sig[:, 128:256])
                            nc.vector.tensor_add(cv[:], cv[:], ig[:])
                        else:
                            nc.vector.tensor_copy(cv[:], ig[:])
                        tch = wpool.tile([128, 128], dt.float32, name="tch")
                        nc.scalar.activation(tch[:], cv[:], mybir.ActivationFunctionType.Tanh)
                        h_sl = wpool.tile([128, 128], dt.bfloat16, name="h_sl")
                        nc.vector.tensor_mul(h_sl[:], sig[:, 256:384], tch[:])
                        nc.sync.dma_start(ccin_p[l % 2], h_sl[:])

                        # post the pair AllGather once both layers are staged
                        if l == plast:
                            nc.gpsimd.collective_compute(
                                "AllGather", mybir.AluOpType.bypass,
                                replica_groups=rg, ins=[ccin_p[:]], outs=[ccout_p[:]],
                            )
                            posted.append((pair, ccout_p))

                # fills AFTER both triggers in program order, so the gather-
                # completion waits never delay a trigger on the GpSimd queue.
                for pair, ccout_p in posted:
                    for lp in pair:
                        if lp not in act_map:
                            continue
                        tl = act_map[lp]
                        # ccout[r, lp%2, p, b] -> hb[lp][tl%8][p, r, b]
                        nc.gpsimd.dma_start(
                            hb[lp][tl % 8][:],
                            ccout_p[:, lp % 2, :, :].transpose((1, 0, 2)),
                        )

    nc.compile()
    return nc


_CACHED = {}


def _get_nc(t_run):
    if t_run in _CACHED:
        return _CACHED[t_run]
    import concourse.bass as bass  # noqa: PLC0415
    import concourse.tile as tile  # noqa: PLC0415
    from concourse import bacc, mybir  # noqa: PLC0415

    nc = bacc.Bacc("TRN2", target_bir_lowering=False, debug=False,
                   num_devices=NCORES)
    _build(nc, tile, mybir, t_run)
    _CACHED[t_run] = nc
    return nc


def _assemble(results, t_run):
    """results[j]["logits"]: [t, VSL, b] -> full [B, t_run, V]."""
    logits = np.empty((B, t_run, V), np.float32)
    for j in range(NCORES):
        out = np.asarray(results[j]["logits"], np.float32)  # [t, vv, b]
        logits[:, :, j * VSL:(j + 1) * VSL] = out.transpose(2, 0, 1)
    return logits


def kernel(idx, embed, Wx, Wh, b, Wout, _t_run=T):
    from concourse.bass_utils import run_bass_kernel_spmd  # noqa: PLC0415

    t_run = _t_run
    in_maps = _host_prep(idx, embed, Wx, Wh, b, Wout, t_run)
    nc = _get_nc(t_run)
    res = run_bass_kernel_spmd(nc, in_maps, core_ids=list(range(NCORES)))
    return _assemble(res.results, t_run)


# revision 8
# speedup vs baseline: 1.7344x; 1.0445x over previous
"""CharLSTM (B=128, T=256, V=256, D=1024, L=4) on 8 trn2 NeuronCores.

Strategy: tensor-parallel over the 4*D gate dimension (each core owns a
512-wide slice: 128 columns of each gate, mtile order [i, f, o, g]).
Per time step, each core computes its z slice with bf16 matmuls
(fp32 PSUM accumulation), applies the LSTM cell elementwise for its
128-wide h/c slice, and the 8 h slices are AllGathered (in two
layer-pairs per tick) so every core has the full h vectors for the
next step / next layer.  Layers run with a wavefront skew of SKEW
ticks so the AllGather latency hides under independent matmuls.

Gathered h state lives in per-(layer, slot) SBUF tiles (8 rotating
slots indexed by t%8) so the Tile dependency tracker sees the
x-part matmuls of tick N+1 as independent of tick N's AllGather --
a single big h buffer falsely serializes every tick behind the
newest gather.

Layer-0's x contribution uses a one-hot matmul against G0 = embed @ Wx[0]
(V=256 -> K=256 contraction instead of K=1024), with the one-hot built
on the host from idx (pure int relabeling of the input).

The output projection (h3 @ Wout) is sharded over the vocab dim:
each core holds a 32-column slice of Wout and projects every step
(2-tick delayed so it only uses settled gathers); the host
concatenates the 8 vocab slices.
"""

import numpy as np
import ml_dtypes

B, T, V, D, L = 128, 256, 256, 1024, 4
NCORES = 8
SKEW = 2
VSL = V // NCORES  # per-core vocab slice (32)
PROJ_DELAY = SKEW * 3 + 2  # proj of step t runs at tick t + PROJ_DELAY
BF16 = ml_dtypes.bfloat16

# mtile order [i, f, o, g]; reference gate column blocks are i,f,g,o.
GATE_STARTS = [0, D, 3 * D, 2 * D]


def _gcols(j, m):
    s = GATE_STARTS[m] + j * 128
    return slice(s, s + 128)


def _host_prep(idx, embed, Wx, Wh, b, Wout, t_run):
    """Build per-core input maps (numpy)."""
    nw = t_run // 4
    idx = np.asarray(idx)
    embed = np.asarray(embed, np.float32)
    Wx = np.asarray(Wx, np.float32)
    Wh = np.asarray(Wh, np.float32)
    b = np.asarray(b, np.float32)
    Wout = np.asarray(Wout, np.float32)

    # embt[p, k, v] = embed[v, k*128+p]
    embt = np.ascontiguousarray(
        embed.T.reshape(8, 128, V).transpose(1, 0, 2)).astype(BF16)
    # one-hot: oh[w, p, c, kk, bb] = (idx[bb, 4w+kk] == c*128+p)
    ids = idx[:, :t_run]  # [B, t_run]
    onehot = (ids[None, :, :] == np.arange(V)[:, None, None])  # [V, B, t]
    oh_full = onehot.reshape(2, 128, B, nw, 4)
    oh = np.ascontiguousarray(
        oh_full.transpose(3, 1, 0, 4, 2)).astype(BF16)  # [w, p, c, kk, bb]

    in_maps = []
    for j in range(NCORES):
        wx_j = np.empty((L, 128, 8, 4, 128), np.float32)
        wh_j = np.empty((L, 128, 8, 4, 128), np.float32)
        bias_j = np.empty((128, L, 4), np.float32)
        for l in range(L):
            for m in range(4):
                cols = _gcols(j, m)
                # [1024, 128] -> [8, 128p, 128mm] -> assign [p, k, mm]
                wx_j[l, :, :, m, :] = Wx[l][:, cols].reshape(8, 128, 128).transpose(1, 0, 2)
                wh_j[l, :, :, m, :] = Wh[l][:, cols].reshape(8, 128, 128).transpose(1, 0, 2)
                bias_j[:, l, m] = b[l][cols]
        # wout[p, k, vv] = Wout[k*128+p, j*32+vv]
        wout_j = np.ascontiguousarray(
            Wout[:, j * VSL:(j + 1) * VSL].reshape(8, 128, VSL).transpose(1, 0, 2)
        ).astype(BF16)
        in_maps.append({
            "wx": wx_j.astype(BF16),
            "wh": wh_j.astype(BF16),
            "bias": bias_j,
            "embt": embt,
            "wout": wout_j,
            "oh": oh,
        })
    return in_maps


def _build(nc, tile, mybir, t_run):
    """Emit the SPMD program for one core (identical on all cores)."""
    dt = mybir.dt
    nw = t_run // 4

    wx_ext = nc.dram_tensor("wx", [L, 128, 8, 4, 128], dt.bfloat16, kind="ExternalInput")
    wh_ext = nc.dram_tensor("wh", [L, 128, 8, 4, 128], dt.bfloat16, kind="ExternalInput")
    bias_ext = nc.dram_tensor("bias", [128, L, 4], dt.float32, kind="ExternalInput")
    embt_ext = nc.dram_tensor("embt", [128, 8, V], dt.bfloat16, kind="ExternalInput")
    wout_ext = nc.dram_tensor("wout", [128, 8, VSL], dt.bfloat16, kind="ExternalInput")
    oh_ext = nc.dram_tensor("oh", [nw, 128, 2, 4, 128], dt.bfloat16, kind="ExternalInput")
    out_ext = nc.dram_tensor("logits", [t_run, VSL, 128], dt.float32, kind="ExternalOutput")

    rg = [list(range(NCORES))]

    # gate index m in mtile order [i, f, o, g]; zf regions hold [i, f, g]
    FIG = [(1, 1), (0, 0), (3, 2)]  # (mtile m, zf region) in h-matmul order f,i,g
    M_O = 2

    with tile.TileContext(nc) as tc:
        with (
            tc.tile_pool(name="const", bufs=1) as cpool,
            tc.tile_pool(name="state", bufs=1) as spool,
            tc.tile_pool(name="work", bufs=6) as wpool,
            tc.tile_pool(name="ohp", bufs=2) as ohpool,
            tc.tile_pool(name="zfpsum", bufs=4, space="PSUM") as zfpsum,
            tc.tile_pool(name="zopsum", bufs=3, space="PSUM") as zopsum,
            tc.tile_pool(name="ppsum", bufs=1, space="PSUM") as ppsum,
            tc.tile_pool(name="ccin", bufs=3, space="DRAM") as ccin_pool,
            tc.tile_pool(name="ccout", bufs=3, space="DRAM") as ccout_pool,
        ):
            # ---- resident tiles ----
            wx_s = cpool.tile([128, L, 8, 4, 128], dt.bfloat16)
            wh_s = cpool.tile([128, L, 8, 4, 128], dt.bfloat16)
            bias_s = cpool.tile([128, L, 4], dt.float32)
            embt_s = cpool.tile([128, 8, V], dt.bfloat16)
            wout_s = cpool.tile([128, 8, VSL], dt.bfloat16)
            g0_s = cpool.tile([128, 2, 4, 128], dt.bfloat16)
            # gathered h state: separate tile per (layer, slot) so tick N+1's
            # x-part reads (old slots) don't serialize behind tick N's fill.
            hb = [[cpool.tile([128, 8, 128], dt.bfloat16, name=f"hb{l}_{s}")
                   for s in range(8)] for l in range(L)]
            c_s = [spool.tile([128, 128], dt.float32, name=f"c{l}") for l in range(L)]

            for l in range(L):
                nc.sync.dma_start(wx_s[:, l], wx_ext[l])
                nc.sync.dma_start(wh_s[:, l], wh_ext[l])
            nc.sync.dma_start(bias_s[:], bias_ext[:])
            nc.sync.dma_start(embt_s[:], embt_ext[:])
            nc.sync.dma_start(wout_s[:], wout_ext[:])

            # h(-1) = 0 lives in slot 7 (t=-1 mod 8)
            for l in range(L):
                nc.vector.memset(hb[l][7][:], 0.0)

            # ---- G0 = embed @ Wx[0] (slice), bf16 ----
            for c in range(2):
                pg = zfpsum.tile([128, 512], dt.float32, tag="zf")
                for k in range(8):
                    nc.tensor.matmul(
                        pg[:],
                        embt_s[:, k, c * 128:(c + 1) * 128],
                        wx_s[:, 0, k],
                        start=(k == 0), stop=(k == 7),
                    )
                nc.vector.tensor_copy(g0_s[:, c], pg[:].rearrange("p (m n) -> p m n", m=4))

            # ---- main loop over ticks ----
            total_ticks = t_run + PROJ_DELAY
            oh_tiles = {}
            for tau in range(total_ticks):
                active = [(l, tau - SKEW * l) for l in range(L)]
                active = [(l, t) for (l, t) in active if 0 <= t < t_run]
                act_map = dict(active)

                # one-hot window prefetch for layer 0 (2 ticks ahead of use)
                for w in range(nw):
                    if max(0, w * 4 - 2) == tau:
                        oht = ohpool.tile([128, 2, 4, 128], dt.bfloat16, name="oh_t")
                        nc.sync.dma_start(oht[:], oh_ext[w])
                        oh_tiles[w] = oht
                oh_t = oh_tiles.get(tau // 4)

                # ---- projection of h3 (settled gathers only) ----
                tp = tau - PROJ_DELAY
                if 0 <= tp < t_run:
                    pp = ppsum.tile([VSL, 128], dt.float32, name="pp", tag="pp")
                    hsrc = hb[3][tp % 8]
                    for k in range(8):
                        nc.tensor.matmul(
                            pp[:], wout_s[:, k], hsrc[:, k, :],
                            start=(k == 0), stop=(k == 7),
                        )
                    lg = wpool.tile([VSL, 128], dt.float32, name="lg")
                    nc.vector.tensor_copy(lg[:], pp[:])
                    nc.sync.dma_start(out_ext[tp], lg[:])

                # ---- phase 1: x-part matmuls (i,f,g gates) of ALL active ----
                # layers (depend only on gathers >= 2 ticks old). The o-gate
                # x-part runs in phase 2 so the o PSUM bank's lifetime stays
                # short (PSUM budget: 4 zf + 3 zo + 1 pp = 8 banks).
                zps = {}
                for l, t in active:
                    zf = zfpsum.tile([128, 384], dt.float32, name="zf", tag="zf")
                    zps[l] = zf
                    for m, r in FIG:
                        if l == 0:
                            xmms = [(g0_s[:, c, m], oh_t[:, c, t % 4, :])
                                    for c in range(2)]
                        else:
                            hsrc = hb[l - 1][t % 8]
                            xmms = [(wx_s[:, l, k, m], hsrc[:, k, :])
                                    for k in range(8)]
                        n = len(xmms)
                        for i, (lhsT, rhs) in enumerate(xmms):
                            nc.tensor.matmul(
                                zf[:, r * 128:(r + 1) * 128], lhsT, rhs,
                                start=(i == 0), stop=(t == 0 and i == n - 1),
                            )

                # ---- phase 2: per layer: h-part matmuls + cell + stage ----
                # pair B (layers 2,3) first so its AllGather posts early; the
                # other pair's h-matmuls + next tick's x-parts cover each
                # gather's flight time symmetrically.
                ccinA = ccin_pool.tile([2, 128, 128], dt.bfloat16, name="ccinA")
                ccinB = ccin_pool.tile([2, 128, 128], dt.bfloat16, name="ccinB")
                ccoutA = ccout_pool.tile([NCORES, 2, 128, 128], dt.bfloat16,
                                         addr_space="Shared", name="ccoutA")
                ccoutB = ccout_pool.tile([NCORES, 2, 128, 128], dt.bfloat16,
                                         addr_space="Shared", name="ccoutB")
                pairs = [((2, 3), ccinB, ccoutB), ((0, 1), ccinA, ccoutA)]
                posted = []

                for pair, ccin_p, ccout_p in pairs:
                    plast = max([l for l in pair if l in act_map], default=None)
                    for l in pair:
                        if l not in act_map:
                            continue
                        t = act_map[l]
                        zf = zps[l]
                        zo = zopsum.tile([128, 128], dt.float32, name="zo", tag="zo")
                        # o-gate x-part (needs only old gathers; runs while
                        # this pair's fresh fill is still in flight)
                        if l == 0:
                            xmms = [(g0_s[:, c, M_O], oh_t[:, c, t % 4, :])
                                    for c in range(2)]
                        else:
                            hsrc_x = hb[l - 1][t % 8]
                            xmms = [(wx_s[:, l, k, M_O], hsrc_x[:, k, :])
                                    for k in range(8)]
                        n = len(xmms)
                        for i, (lhsT, rhs) in enumerate(xmms):
                            nc.tensor.matmul(
                                zo[:], lhsT, rhs,
                                start=(i == 0), stop=(t == 0 and i == n - 1),
                            )
                        if t > 0:
                            hsrc = hb[l][(t - 1) % 8]
                            for m, r in FIG:
                                for k in range(8):
                                    nc.tensor.matmul(
                                        zf[:, r * 128:(r + 1) * 128],
                                        wh_s[:, l, k, m], hsrc[:, k, :],
                                        start=False, stop=(k == 7),
                                    )
                            for k in range(8):
                                nc.tensor.matmul(
                                    zo[:], wh_s[:, l, k, M_O], hsrc[:, k, :],
                                    start=False, stop=(k == 7),
                                )

                        # ---- LSTM cell (z^T layout: partition = gate dim) ----
                        # zf regions: i=[0:128], f=[128:256], g=[256:384];
                        # ACT order f,i,g then o; o-gate lives in its own
                        # PSUM bank so its ACT overlaps the c-update chain.
                        si = wpool.tile([128, 128], dt.float32, name="si")
                        sf = wpool.tile([128, 128], dt.float32, name="sf")
                        tg = wpool.tile([128, 128], dt.float32, name="tg")
                        so = wpool.tile([128, 128], dt.float32, name="so")
                        cv = c_s[l]
                        if t > 0:
                            nc.scalar.activation(
                                sf[:], zf[:, 128:256],
                                mybir.ActivationFunctionType.Sigmoid,
                                bias=bias_s[:, l, 1:2],
                            )
                        nc.scalar.activation(
                            si[:], zf[:, 0:128],
                            mybir.ActivationFunctionType.Sigmoid,
                            bias=bias_s[:, l, 0:1],
                        )
                        nc.scalar.activation(
                            tg[:], zf[:, 256:384],
                            mybir.ActivationFunctionType.Tanh,
                            bias=bias_s[:, l, 3:4],
                        )
                        nc.scalar.activation(
                            so[:], zo[:],
                            mybir.ActivationFunctionType.Sigmoid,
                            bias=bias_s[:, l, 2:3],
                        )
                        ig = wpool.tile([128, 128], dt.float32, name="ig")
                        nc.vector.tensor_mul(ig[:], si[:], tg[:])
                        if t > 0:
                            nc.vector.tensor_mul(cv[:], cv[:], sf[:])
                            nc.vector.tensor_add(cv[:], cv[:], ig[:])
                        else:
                            nc.vector.tensor_copy(cv[:], ig[:])
                        tch = wpool.tile([128, 128], dt.float32, name="tch")
                        nc.scalar.activation(tch[:], cv[:], mybir.ActivationFunctionType.Tanh)
                        h_sl = wpool.tile([128, 128], dt.bfloat16, name="h_sl")
                        nc.vector.tensor_mul(h_sl[:], so[:], tch[:])
                        nc.sync.dma_start(ccin_p[l % 2], h_sl[:])

                        # post the pair AllGather once both layers are staged
                        if l == plast:
                            nc.gpsimd.collective_compute(
                                "AllGather", mybir.AluOpType.bypass,
                                replica_groups=rg, ins=[ccin_p[:]], outs=[ccout_p[:]],
                            )
                            posted.append((pair, ccout_p))

                # fills AFTER both triggers in program order, so the gather-
                # completion waits never delay a trigger on the GpSimd queue.
                # First layer of each pair (consumed first next tick) is
                # split into two half-fills for lower transfer latency.
                for pair, ccout_p in posted:
                    first = True
                    for lp in pair:
                        if lp not in act_map:
                            continue
                        tl = act_map[lp]
                        dst = hb[lp][tl % 8]
                        src = ccout_p[:, lp % 2, :, :]
                        # ccout[r, lp%2, p, b] -> hb[lp][tl%8][p, r, b]
                        if first:
                            nc.gpsimd.dma_start(
                                dst[:, 0:4, :], src[0:4].transpose((1, 0, 2)))
                            nc.gpsimd.dma_start(
                                dst[:, 4:8, :], src[4:8].transpose((1, 0, 2)))
                            first = False
                        else:
                            nc.gpsimd.dma_start(dst[:], src.transpose((1, 0, 2)))

    nc.compile()
    return nc


_CACHED = {}


def _get_nc(t_run):
    if t_run in _CACHED:
        return _CACHED[t_run]
    import concourse.bass as bass  # noqa: PLC0415
    import concourse.tile as tile  # noqa: PLC0415
    from concourse import bacc, mybir  # noqa: PLC0415

    nc = bacc.Bacc("TRN2", target_bir_lowering=False, debug=False,
                   num_devices=NCORES)
    _build(nc, tile, mybir, t_run)
    _CACHED[t_run] = nc
    return nc


def _assemble(results, t_run):
    """results[j]["logits"]: [t, VSL, b] -> full [B, t_run, V]."""
    logits = np.empty((B, t_run, V), np.float32)
    for j in range(NCORES):
        out = np.asarray(results[j]["logits"], np.float32)  # [t, vv, b]
        logits[:, :, j * VSL:(j + 1) * VSL] = out.transpose(2, 0, 1)
    return logits


def kernel(idx, embed, Wx, Wh, b, Wout, _t_run=T):
    from concourse.bass_utils import run_bass_kernel_spmd  # noqa: PLC0415

    t_run = _t_run
    in_maps = _host_prep(idx, embed, Wx, Wh, b, Wout, t_run)
    nc = _get_nc(t_run)
    res = run_bass_kernel_spmd(nc, in_maps, core_ids=list(range(NCORES)))
    return _assemble(res.results, t_run)
